# revision 14
# baseline (speedup 1.0000x reference)
"""Chunked multi-head attention (CMHA) Trainium2 kernel.

Distribution: 8 NeuronCores = data-parallel over batch (2) x tensor-parallel
over heads (4 heads/core).  Each core computes, for its (batch, head-group):
qkv projection, low-rank off-diagonal scores t2c[l,n]*c2t[n,c], direct
diagonal-chunk scores, multiplicative causal mask (masked scores -> 0, so
exp(0)=1 still contributes), softmax without max-subtraction (|S|/32 <= ~2),
PV, and a partial o_proj.  Host sums the 4 partial o_proj outputs per batch
and adds b_o.

Key algebraic facts used:
  - chunk-mean pooling commutes with the qkv projection, so q_pool/k_pool/
    chunk-sums of V come from projecting a host-pooled x_pool (tiny matmuls).
  - masked (upper) region contributes exp(0)=1 weights: its PV contribution is
    128*(x_pool@Wv+bv) per masked chunk and its Z contribution is 128, both
    obtained from the pooled projection and one "suffix" matmul.
  - scores are built transposed (S_T[c, l]) so the PV matmul needs no
    transposes: lhsT = V_chunk, rhs = exp(S_T) pieces.
  - 1/Z is computed as exp(-ln Z) on the scalar engine (Ln and Exp share one
    activation table set), with the Z row broadcast across partitions by a
    K=1 PE matmul.

fp32r matmul ISA restrictions honoured: moving operand innermost count even,
dst innermost count even + 8B aligned + start_partition 0.
"""

import numpy as np

B, L, D = 2, 2048, 1024
H, HD, CH, NC = 16, 64, 128, 16  # heads, head_dim, chunk size, num chunks
HPC = 4  # heads per core
NCORES = 8
SCALE = 32.0  # sqrt(D)

_CACHE = {}


def _split_banked(lo, hi, bank=512):
    """Split [lo, hi) into pieces that do not cross `bank` boundaries."""
    out = []
    while lo < hi:
        nxt = min(hi, (lo // bank + 1) * bank)
        out.append((lo, nxt))
        lo = nxt
    return out


def _split_multiwaits(nc, limit=1):
    """Walrus in this container rejects instructions with more than one sync
    wait.  Move excess waits onto same-engine InstNoOp carriers inserted just
    before the offending instruction (engine streams execute in block order,
    so this is semantically identical)."""
    import concourse.mybir as mybir

    f = nc.m.functions[0]
    for blk in f.blocks:
        il = blk.instructions
        idx = 0
        while idx < len(il):
            inst = il[idx]
            si = getattr(inst, "sync_info", None)
            if si is None or not hasattr(si, "on_wait"):
                idx += 1
                continue
            waits = list(si.on_wait)
            if len(waits) <= limit:
                idx += 1
                continue
            eng = inst.engine
            keep, extra = waits[:limit], waits[limit:]
            nops = []
            for w in extra:
                n = nc.engines[eng].nop(hint="waitsplit").ins
                for b2 in f.blocks:
                    l2 = b2.instructions
                    for k in range(len(l2) - 1, -1, -1):
                        if l2[k] is n or l2[k].name == n.name:
                            l2.pop(k)
                            break
                n.sync_info = mybir.SyncInfo(on_wait=[w], on_update=[])
                nops.append(n)
            inst.sync_info = mybir.SyncInfo(on_wait=keep, on_update=list(si.on_update))
            for j, n in enumerate(nops):
                il.insert(idx + j, n)
            idx += len(nops) + 1
    return nc


def _build_nc():
    import concourse.bass as bass
    import concourse.mybir as mybir
    import concourse.tile as tile

    f32 = mybir.dt.float32
    f32r = mybir.dt.float32r
    AF = mybir.ActivationFunctionType
    Alu = mybir.AluOpType

    nc = bass.Bass(trn_type="TRN2", target_bir_lowering=False, debug=False)

    # ---- DRAM I/O ----
    d_xt = nc.dram_tensor("xt", [D, L], f32r, kind="ExternalInput")
    d_xtp = nc.dram_tensor("xtp", [D, NC], f32r, kind="ExternalInput")
    d_wqk = nc.dram_tensor("wqk", [D, 512], f32r, kind="ExternalInput")
    d_bqk = nc.dram_tensor("bqk", [128, 4], f32, kind="ExternalInput")
    d_wv = nc.dram_tensor("wv", [D, 256], f32r, kind="ExternalInput")
    d_bvb = nc.dram_tensor("bvb", [128, 256], f32, kind="ExternalInput")
    d_bvo = nc.dram_tensor("bvo", [1, 4 * 66], f32r, kind="ExternalInput")
    d_wo = nc.dram_tensor("wo", [256, D], f32r, kind="ExternalInput")
    d_tri = nc.dram_tensor("trimask", [128, 128], f32, kind="ExternalInput")
    d_bdm = nc.dram_tensor("bdmask", [NC, L], f32, kind="ExternalInput")
    d_suf = nc.dram_tensor("suf", [NC, L], f32r, kind="ExternalInput")
    d_out = nc.dram_tensor("out", [L, D], f32, kind="ExternalOutput")

    with tile.TileContext(nc) as tc:
        with tc.tile_pool(name="pers", bufs=1) as pers:
            # ---- persistent SBUF ----
            wqk_sb = pers.tile([128, 8, 512], f32r, tag="wqk")
            wv_sb = pers.tile([128, 8, 256], f32r, tag="wv")
            wo_sb = pers.tile([128, 2, D], f32r, tag="wo")
            xtp_sb = pers.tile([128, 8, NC], f32r, tag="xtp")
            bqk_sb = pers.tile([128, 4], f32, tag="bqk")
            bvb_sb = pers.tile([128, 256], f32, tag="bvb")
            bvo_sb = pers.tile([1, 4 * 66], f32r, tag="bvo")
            qk_sb = pers.tile([128, 4, L], f32r, tag="qk")
            v_sb = pers.tile([128, NC, HPC, 65], f32r, tag="v")
            pool_sb = pers.tile([128, 4, 32], f32r, tag="pool")
            cs_sb = pers.tile([NC, HPC, 65], f32r, tag="cs")
            ones_sb = pers.tile([1, 64], f32r, tag="ones")

            # ---- input DMAs (xt is scoped: freed after the projections) ----
            nc.sync.dma_start(wqk_sb[:, :, :], d_wqk.rearrange("(a p) e -> p a e", p=128))
            nc.sync.dma_start(wv_sb[:, :, :], d_wv.rearrange("(a p) e -> p a e", p=128))
            nc.sync.dma_start(wo_sb[:, :, :], d_wo.rearrange("(j p) d -> p j d", p=128))
            nc.sync.dma_start(xtp_sb[:, :, :], d_xtp.rearrange("(a p) n -> p a n", p=128))
            nc.sync.dma_start(bqk_sb[:, :], d_bqk[:, :])
            nc.sync.dma_start(bvb_sb[:, :], d_bvb[:, :])
            nc.sync.dma_start(bvo_sb[:, :], d_bvo[:, :])

            nc.gpsimd.memset(ones_sb[:, :].bitcast(f32), 1.0)
            nc.gpsimd.memset(pool_sb[:, :, :].bitcast(f32), 0.0)
            nc.gpsimd.memset(v_sb[:, :, :, 64:65].bitcast(f32), 1.0)

            # ---- phase A: projections ----
            with tc.tile_pool(name="pxt", bufs=1) as pxt:
                xt_sb = pxt.tile([128, 8, L], f32r, tag="xt")
                xt_r = d_xt.rearrange("(a p) l -> p a l", p=128)
                for a in range(8):
                    nc.sync.dma_start(xt_sb[:, a, :], xt_r[:, a, :])

                # Q,K in [e, l] layout: out = wqk.T @ xt
                with tc.tile_pool(
                    name="ps_qk", bufs=2, space=bass.MemorySpace.PSUM
                ) as ps_qk:
                    for j in range(4):
                        ps = ps_qk.tile([128, L], f32, tag="qk")
                        for p0, p1 in _split_banked(0, L):
                            for a in range(8):
                                nc.tensor.matmul(
                                    ps[:, p0:p1],
                                    wqk_sb[:, a, 128 * j : 128 * (j + 1)],
                                    xt_sb[:, a, p0:p1],
                                    start=(a == 0),
                                    stop=(a == 7),
                                )
                        nc.vector.tensor_scalar_add(
                            qk_sb[:, j, :], ps[:, :], bqk_sb[:, j : j + 1]
                        )

                # V in [l, e] layout: out = xt.T-chunk @ wv  (+bias via bvb)
                with tc.tile_pool(
                    name="ps_v", bufs=3, space=bass.MemorySpace.PSUM
                ) as ps_v, tc.tile_pool(
                    name="ps_sm", bufs=1, space=bass.MemorySpace.PSUM
                ) as ps_sm:
                    for ci in range(NC):
                        psv = ps_v.tile([128, 256], f32, tag="v")
                        for a in range(8):
                            nc.tensor.matmul(
                                psv[:, :],
                                xt_sb[:, a, 128 * ci : 128 * (ci + 1)],
                                wv_sb[:, a, :],
                                start=(a == 0),
                                stop=(a == 7),
                            )
                        nc.vector.tensor_tensor(
                            v_sb[:, ci, :, 0:64],
                            psv.rearrange("p (h e) -> p h e", h=HPC),
                            bvb_sb.rearrange("p (h e) -> p h e", h=HPC),
                            Alu.add,
                        )

                    # pooled projections: q_pool/k_pool [e, n]
                    psp = ps_sm.tile([128, 4, NC], f32, tag="pool")
                    for j in range(4):
                        for a in range(8):
                            nc.tensor.matmul(
                                psp[:, j, :],
                                wqk_sb[:, a, 128 * j : 128 * (j + 1)],
                                xtp_sb[:, a, :],
                                start=(a == 0),
                                stop=(a == 7),
                            )
                    for j in range(4):
                        nc.scalar.activation(
                            pool_sb[:, j, 0:16],
                            psp[:, j, :],
                            AF.Identity,
                            bias=bqk_sb[:, j : j + 1],
                        )

                    # chunk-sums of V_aug: cs = 128*(x_pool@wv+bv | 1) per head
                    # (66-wide psum lanes keep the fp32r dst rules satisfied)
                    psc = ps_sm.tile([NC, HPC, 66], f32, tag="cs")
                    for h in range(HPC):
                        nc.tensor.matmul(
                            psc[:, h, :],
                            ones_sb[:, 0:16],
                            bvo_sb[:, 66 * h : 66 * (h + 1)],
                            start=True,
                            stop=False,
                        )
                        for a in range(8):
                            nc.tensor.matmul(
                                psc[:, h, 0:64],
                                xtp_sb[:, a, :],
                                wv_sb[:, a, 64 * h : 64 * (h + 1)],
                                start=False,
                                stop=(a == 7),
                            )
                    for h in range(HPC):
                        nc.scalar.activation(
                            cs_sb[:, h, :], psc[:, h, 0:65], AF.Copy, scale=128.0
                        )

            # phase-B tensors (allocated after xt is freed)
            tri_sb = pers.tile([128, 128], f32, tag="tri")
            bdm_sb = pers.tile([NC, L], f32, tag="bdm")
            suf_sb = pers.tile([NC, L], f32r, tag="suf")
            t2c_sb = pers.tile([128, L], f32r, tag="t2c")
            c2t_sb = pers.tile([128, L], f32r, tag="c2t")
            oh_sb = pers.tile([128, 2, L], f32r, tag="oh")
            nc.sync.dma_start(tri_sb[:, :], d_tri[:, :])
            nc.sync.dma_start(bdm_sb[:, :], d_bdm[:, :])
            nc.sync.dma_start(suf_sb[:, :], d_suf[:, :])

            # t2c[n, l] / c2t[n, l] per head.  fp32r matmuls must write psum
            # at partition 0, so compute each head's [16, L] there, then DMA
            # SBUF->SBUF into the partition-packed layout (head h at
            # partitions 32h..32h+15) that the S_T matmuls read.
            with tc.tile_pool(
                name="ps_tc", bufs=2, space=bass.MemorySpace.PSUM
            ) as ps_tc, tc.tile_pool(name="tcs", bufs=4) as tcs:
                for h in range(HPC):
                    rb = 64 * (h % 2)
                    qblk, kblk = h // 2, 2 + h // 2
                    pst = ps_tc.tile([NC, L], f32, tag="tc")
                    psc2 = ps_tc.tile([NC, L], f32, tag="tc")
                    for p0, p1 in _split_banked(0, L):
                        nc.tensor.matmul(
                            pst[:, p0:p1],
                            pool_sb[rb : rb + 64, kblk, 0:16],
                            qk_sb[rb : rb + 64, qblk, p0:p1],
                            start=True,
                            stop=True,
                        )
                        nc.tensor.matmul(
                            psc2[:, p0:p1],
                            pool_sb[rb : rb + 64, qblk, 0:16],
                            qk_sb[rb : rb + 64, kblk, p0:p1],
                            start=True,
                            stop=True,
                        )
                    st1 = tcs.tile([NC, L], f32r, tag="tcs")
                    st2 = tcs.tile([NC, L], f32r, tag="tcs")
                    nc.scalar.activation(st1[:, :], pst[:, :], AF.Copy)
                    nc.vector.tensor_tensor(st2[:, :], psc2[:, :], bdm_sb[:, :], Alu.mult)
                    nc.sync.dma_start(t2c_sb[32 * h : 32 * h + 16, :], st1[:, :])
                    nc.sync.dma_start(c2t_sb[32 * h : 32 * h + 16, :], st2[:, :])

            # ---- phase B: per (head, l-half) score/softmax/PV ----
            with tc.tile_pool(
                name="ps_s", bufs=2, space=bass.MemorySpace.PSUM
            ) as ps_s, tc.tile_pool(
                name="ps_out", bufs=2, space=bass.MemorySpace.PSUM
            ) as ps_out, tc.tile_pool(
                name="pexp", bufs=2
            ) as pexp, tc.tile_pool(name="zp", bufs=2) as zp, tc.tile_pool(
                name="rzbp", bufs=1
            ) as rzbp:
                for h in range(HPC):
                    rb = 64 * (h % 2)
                    qblk, kblk = h // 2, 2 + h // 2
                    for half in range(2):
                        l0 = 1024 * half
                        chunks = [n for n in range(NC) if n * CH < l0 + 1024]
                        pso = ps_out.tile([65, 1024], f32, tag="out")
                        for n in chunks:
                            lstart = max(n * CH, l0)
                            span = l0 + 1024 - lstart
                            has_diag = n * CH >= l0
                            pss = ps_s.tile([128, span], f32, tag="s")
                            if has_diag:
                                nc.tensor.matmul(
                                    pss[:, 0:128],
                                    qk_sb[rb : rb + 64, kblk, n * CH : n * CH + 128],
                                    qk_sb[rb : rb + 64, qblk, n * CH : n * CH + 128],
                                    start=True,
                                    stop=True,
                                )
                                off0 = 128
                            else:
                                off0 = 0
                            for p0, p1 in _split_banked(off0, span):
                                nc.tensor.matmul(
                                    pss[:, p0:p1],
                                    c2t_sb[32 * h : 32 * h + 16, n * CH : n * CH + 128],
                                    t2c_sb[32 * h : 32 * h + 16, lstart + p0 : lstart + p1],
                                    start=True,
                                    stop=True,
                                    tile_position=(32 * h, 0),
                                )
                            if has_diag:
                                nc.vector.tensor_tensor(
                                    pss[:, 0:128], pss[:, 0:128], tri_sb[:, :], Alu.mult
                                )
                            pt = pexp.tile([128, span], f32r, tag="p")
                            nc.scalar.activation(
                                pt[:, :], pss[:, :], AF.Exp, scale=1.0 / SCALE
                            )
                            # PV accumulate
                            oofs = lstart - l0
                            first = n == chunks[0]
                            for p0, p1 in _split_banked(oofs, 1024):
                                nc.tensor.matmul(
                                    pso[:, p0:p1],
                                    v_sb[:, n, h, :],
                                    pt[:, p0 - oofs : p1 - oofs],
                                    start=first,
                                    stop=False,
                                    skip_group_check=True,
                                )
                        # masked-suffix contribution + Z count
                        for p0, p1 in _split_banked(0, 1024):
                            nc.tensor.matmul(
                                pso[:, p0:p1],
                                cs_sb[:, h, :],
                                suf_sb[:, l0 + p0 : l0 + p1],
                                start=False,
                                stop=True,
                                skip_group_check=True,
                            )
                        # 1/Z = exp(-ln Z): ln of the Z row, K=1 PE broadcast,
                        # exp(-x) evacuates the broadcast to SBUF.
                        lnz = zp.tile([1, 1024], f32r, tag="lnz")
                        nc.scalar.activation(lnz[:, :], pso[64:65, :], AF.Ln)
                        rzb = ps_s.tile([64, 1024], f32, tag="s")
                        for p0, p1 in _split_banked(0, 1024):
                            nc.tensor.matmul(
                                rzb[:, p0:p1],
                                ones_sb[:, :],
                                lnz[:, p0:p1],
                                start=True,
                                stop=True,
                            )
                        rzb_sb = rzbp.tile([64, 1024], f32, tag="rzb")
                        nc.scalar.activation(rzb_sb[:, :], rzb[:, :], AF.Exp, scale=-1.0)
                        nc.vector.tensor_tensor(
                            oh_sb[rb : rb + 64, h // 2, l0 : l0 + 1024],
                            pso[0:64, :],
                            rzb_sb[:, :],
                            Alu.mult,
                        )

            # ---- phase C: partial o_proj ----
            out_r = d_out.rearrange("(t p) d -> p t d", p=128)
            with tc.tile_pool(
                name="ps_o", bufs=2, space=bass.MemorySpace.PSUM
            ) as ps_o, tc.tile_pool(name="outp", bufs=2) as outp:
                for lb in range(16):
                    pso2 = ps_o.tile([128, D], f32, tag="o")
                    for p0, p1 in _split_banked(0, D):
                        for j in range(2):
                            nc.tensor.matmul(
                                pso2[:, p0:p1],
                                oh_sb[:, j, 128 * lb : 128 * (lb + 1)],
                                wo_sb[:, j, p0:p1],
                                start=(j == 0),
                                stop=(j == 1),
                            )
                    ob = outp.tile([128, D], f32, tag="ob")
                    nc.scalar.activation(ob[:, 0:512], pso2[:, 0:512], AF.Copy)
                    nc.vector.tensor_copy(ob[:, 512:], pso2[:, 512:])
                    nc.sync.dma_start(out_r[:, lb, :], ob[:, :])

    _split_multiwaits(nc)
    return nc


def _host_prep(inputs):
    """Build the 8 per-core input maps."""
    x = np.asarray(inputs["x"], dtype=np.float32)
    w_qkv = np.asarray(inputs["w_qkv"], dtype=np.float32)
    b_qkv = np.asarray(inputs["b_qkv"], dtype=np.float32)
    w_o = np.asarray(inputs["w_o"], dtype=np.float32)

    tri = np.triu(np.ones((128, 128), dtype=np.float32))
    bdm = np.zeros((NC, L), dtype=np.float32)
    for n in range(NC):
        bdm[n, n * CH : (n + 1) * CH] = 1.0
    suf = np.zeros((NC, L), dtype=np.float32)
    lidx = np.arange(L) // CH
    for n in range(NC):
        suf[n, lidx < n] = 1.0

    in_maps = []
    for c in range(NCORES):
        b, g = c // HPC, c % HPC
        hs = g * HPC  # first global head
        e0 = hs * HD
        qc = slice(e0, e0 + 256)
        kc = slice(D + e0, D + e0 + 256)
        vc = slice(2 * D + e0, 2 * D + e0 + 256)
        xt = np.ascontiguousarray(x[b].T)
        xtp = np.ascontiguousarray(
            x[b].reshape(NC, CH, D).mean(axis=1, dtype=np.float32).T
        )
        wqk = np.ascontiguousarray(np.concatenate([w_qkv[:, qc], w_qkv[:, kc]], axis=1))
        bqk = np.ascontiguousarray(
            np.concatenate([b_qkv[qc], b_qkv[kc]]).reshape(4, 128).T
        )
        wv = np.ascontiguousarray(w_qkv[:, vc])
        bv = b_qkv[vc]
        bvb = np.tile(bv[None, :], (128, 1)).astype(np.float32)
        bvo = np.zeros((1, 4 * 66), dtype=np.float32)
        for h in range(HPC):
            bvo[0, 66 * h : 66 * h + 64] = bv[64 * h : 64 * (h + 1)]
            bvo[0, 66 * h + 64] = 1.0
        wo = np.ascontiguousarray(w_o[e0 : e0 + 256, :])
        in_maps.append(
            {
                "xt": xt,
                "xtp": xtp,
                "wqk": wqk,
                "bqk": bqk,
                "wv": wv,
                "bvb": bvb,
                "bvo": bvo,
                "wo": wo,
                "trimask": tri,
                "bdmask": bdm,
                "suf": suf,
            }
        )
    return in_maps


def run_cores(inputs, trace=False, trace_kwargs=None):
    """Run the SPMD kernel; returns (per-core results, BassKernelResults)."""
    from concourse.bass_utils import run_bass_kernel_spmd

    if "nc" not in _CACHE:
        _CACHE["nc"] = _build_nc()
    nc = _CACHE["nc"]
    in_maps = _host_prep(inputs)
    res = run_bass_kernel_spmd(
        nc,
        in_maps,
        list(range(NCORES)),
        trace=trace,
        **(trace_kwargs or {}),
    )
    return res.results, res


def kernel(**inputs):
    results, _ = run_cores(inputs)
    b_o = np.asarray(inputs["b_o"], dtype=np.float32)
    out = np.zeros((B, L, D), dtype=np.float32)
    for c in range(NCORES):
        out[c // HPC] += results[c]["out"]
    out += b_o
    return out


# revision 15
# speedup vs baseline: 1.1654x; 1.1654x over previous
"""Chunked multi-head attention (CMHA) Trainium2 kernel.

Distribution: 8 NeuronCores = data-parallel over batch (2) x tensor-parallel
over heads (4 heads/core).  Each core computes, for its (batch, head-group):
qkv projection, low-rank off-diagonal scores t2c[l,n]*c2t[n,c], direct
diagonal-chunk scores, multiplicative causal mask (masked scores -> 0, so
exp(0)=1 still contributes), softmax without max-subtraction (|S|/32 <= ~2),
PV, and a partial o_proj.  Host sums the 4 partial o_proj outputs per batch
and adds b_o.

Key algebraic facts used:
  - chunk-mean pooling commutes with the qkv projection, so q_pool/k_pool/
    chunk-sums of V come from projecting a host-pooled x_pool (tiny matmuls).
  - masked (upper) region contributes exp(0)=1 weights: its PV contribution is
    128*(x_pool@Wv+bv) per masked chunk and its Z contribution is 128, both
    obtained from the pooled projection and one "suffix" matmul.
  - scores are built transposed (S_T[c, l]) so the PV matmul needs no
    transposes: lhsT = V_chunk, rhs = exp(S_T) pieces.
  - 1/Z is computed as exp(-ln Z) on the scalar engine (Ln and Exp share one
    activation table set), with the Z row broadcast across partitions by a
    K=1 PE matmul.

fp32r matmul ISA restrictions honoured: moving operand innermost count even,
dst innermost count even + 8B aligned + start_partition 0.
"""

import ml_dtypes
import numpy as np

BF16 = ml_dtypes.bfloat16

B, L, D = 2, 2048, 1024
H, HD, CH, NC = 16, 64, 128, 16  # heads, head_dim, chunk size, num chunks
HPC = 4  # heads per core
NCORES = 8
SCALE = 32.0  # sqrt(D)

_CACHE = {}


def _split_banked(lo, hi, bank=512):
    """Split [lo, hi) into pieces that do not cross `bank` boundaries."""
    out = []
    while lo < hi:
        nxt = min(hi, (lo // bank + 1) * bank)
        out.append((lo, nxt))
        lo = nxt
    return out


def _split_multiwaits(nc, limit=1):
    """Walrus in this container rejects instructions with more than one sync
    wait.  Move excess waits onto same-engine InstNoOp carriers inserted just
    before the offending instruction (engine streams execute in block order,
    so this is semantically identical)."""
    import concourse.mybir as mybir

    f = nc.m.functions[0]
    for blk in f.blocks:
        il = blk.instructions
        idx = 0
        while idx < len(il):
            inst = il[idx]
            si = getattr(inst, "sync_info", None)
            if si is None or not hasattr(si, "on_wait"):
                idx += 1
                continue
            waits = list(si.on_wait)
            if len(waits) <= limit:
                idx += 1
                continue
            eng = inst.engine
            keep, extra = waits[:limit], waits[limit:]
            nops = []
            for w in extra:
                n = nc.engines[eng].nop(hint="waitsplit").ins
                for b2 in f.blocks:
                    l2 = b2.instructions
                    for k in range(len(l2) - 1, -1, -1):
                        if l2[k] is n or l2[k].name == n.name:
                            l2.pop(k)
                            break
                n.sync_info = mybir.SyncInfo(on_wait=[w], on_update=[])
                nops.append(n)
            inst.sync_info = mybir.SyncInfo(on_wait=keep, on_update=list(si.on_update))
            for j, n in enumerate(nops):
                il.insert(idx + j, n)
            idx += len(nops) + 1
    return nc


def _build_nc():
    import concourse.bass as bass
    import concourse.mybir as mybir
    import concourse.tile as tile

    f32 = mybir.dt.float32
    f32r = mybir.dt.float32r
    bf16 = mybir.dt.bfloat16
    AF = mybir.ActivationFunctionType
    Alu = mybir.AluOpType

    nc = bass.Bass(trn_type="TRN2", target_bir_lowering=False, debug=False)

    # ---- DRAM I/O ----
    d_xt = nc.dram_tensor("xt", [D, L], bf16, kind="ExternalInput")
    d_xtp = nc.dram_tensor("xtp", [D, NC], bf16, kind="ExternalInput")
    d_wqk = nc.dram_tensor("wqk", [D, 512], bf16, kind="ExternalInput")
    d_bqk = nc.dram_tensor("bqk", [128, 4], f32, kind="ExternalInput")
    d_wv = nc.dram_tensor("wv", [D, 256], bf16, kind="ExternalInput")
    d_bvb = nc.dram_tensor("bvb", [128, 256], f32, kind="ExternalInput")
    d_bvo = nc.dram_tensor("bvo", [1, 4 * 66], bf16, kind="ExternalInput")
    d_wo = nc.dram_tensor("wo", [256, D], bf16, kind="ExternalInput")
    d_tri = nc.dram_tensor("trimask", [128, 128], f32, kind="ExternalInput")
    d_bdm = nc.dram_tensor("bdmask", [NC, L], f32, kind="ExternalInput")
    d_suf = nc.dram_tensor("suf", [NC, L], bf16, kind="ExternalInput")
    d_out = nc.dram_tensor("out", [L, D], f32, kind="ExternalOutput")

    with tile.TileContext(nc) as tc:
        with tc.tile_pool(name="pers", bufs=1) as pers:
            # ---- persistent SBUF ----
            wqk_sb = pers.tile([128, 8, 512], bf16, tag="wqk")
            wv_sb = pers.tile([128, 8, 256], bf16, tag="wv")
            wo_sb = pers.tile([128, 2, D], bf16, tag="wo")
            xtp_sb = pers.tile([128, 8, NC], bf16, tag="xtp")
            bqk_sb = pers.tile([128, 4], f32, tag="bqk")
            bvb_sb = pers.tile([128, 256], f32, tag="bvb")
            bvo_sb = pers.tile([1, 4 * 66], bf16, tag="bvo")
            qk_sb = pers.tile([128, 4, L], bf16, tag="qk")
            v_sb = pers.tile([128, NC, HPC, 65], bf16, tag="v")
            pool_sb = pers.tile([128, 4, 32], bf16, tag="pool")
            cs_sb = pers.tile([NC, HPC, 65], bf16, tag="cs")
            ones_sb = pers.tile([1, 64], bf16, tag="ones")
            onesr_sb = pers.tile([1, 64], f32r, tag="ones")

            # ---- input DMAs (xt is scoped: freed after the projections) ----
            nc.sync.dma_start(wqk_sb[:, :, :], d_wqk.rearrange("(a p) e -> p a e", p=128))
            nc.sync.dma_start(wv_sb[:, :, :], d_wv.rearrange("(a p) e -> p a e", p=128))
            nc.sync.dma_start(wo_sb[:, :, :], d_wo.rearrange("(j p) d -> p j d", p=128))
            nc.sync.dma_start(xtp_sb[:, :, :], d_xtp.rearrange("(a p) n -> p a n", p=128))
            nc.sync.dma_start(bqk_sb[:, :], d_bqk[:, :])
            nc.sync.dma_start(bvb_sb[:, :], d_bvb[:, :])
            nc.sync.dma_start(bvo_sb[:, :], d_bvo[:, :])

            nc.gpsimd.memset(ones_sb[:, :], 1.0)
            nc.gpsimd.memset(onesr_sb[:, :].bitcast(f32), 1.0)
            nc.gpsimd.memset(pool_sb[:, :, :], 0.0)
            nc.gpsimd.memset(v_sb[:, :, :, 64:65], 1.0)

            # ---- phase A: projections ----
            with tc.tile_pool(name="pxt", bufs=1) as pxt:
                xt_sb = pxt.tile([128, 8, L], bf16, tag="xt")
                xt_r = d_xt.rearrange("(a p) l -> p a l", p=128)
                for a in range(8):
                    nc.sync.dma_start(xt_sb[:, a, :], xt_r[:, a, :])

                # Q,K in [e, l] layout: out = wqk.T @ xt
                with tc.tile_pool(
                    name="ps_qk", bufs=2, space=bass.MemorySpace.PSUM
                ) as ps_qk:
                    for j in range(4):
                        ps = ps_qk.tile([128, L], f32, tag="qk")
                        for p0, p1 in _split_banked(0, L):
                            for a in range(8):
                                nc.tensor.matmul(
                                    ps[:, p0:p1],
                                    wqk_sb[:, a, 128 * j : 128 * (j + 1)],
                                    xt_sb[:, a, p0:p1],
                                    start=(a == 0),
                                    stop=(a == 7),
                                )
                        nc.vector.tensor_scalar_add(
                            qk_sb[:, j, :], ps[:, :], bqk_sb[:, j : j + 1]
                        )

                # V in [l, e] layout: out = xt.T-chunk @ wv  (+bias via bvb)
                with tc.tile_pool(
                    name="ps_v", bufs=3, space=bass.MemorySpace.PSUM
                ) as ps_v, tc.tile_pool(
                    name="ps_sm", bufs=1, space=bass.MemorySpace.PSUM
                ) as ps_sm:
                    for ci in range(NC):
                        psv = ps_v.tile([128, 256], f32, tag="v")
                        for a in range(8):
                            nc.tensor.matmul(
                                psv[:, :],
                                xt_sb[:, a, 128 * ci : 128 * (ci + 1)],
                                wv_sb[:, a, :],
                                start=(a == 0),
                                stop=(a == 7),
                            )
                        nc.vector.tensor_tensor(
                            v_sb[:, ci, :, 0:64],
                            psv.rearrange("p (h e) -> p h e", h=HPC),
                            bvb_sb.rearrange("p (h e) -> p h e", h=HPC),
                            Alu.add,
                        )

                    # pooled projections: q_pool/k_pool [e, n]
                    psp = ps_sm.tile([128, 4, NC], f32, tag="pool")
                    for j in range(4):
                        for a in range(8):
                            nc.tensor.matmul(
                                psp[:, j, :],
                                wqk_sb[:, a, 128 * j : 128 * (j + 1)],
                                xtp_sb[:, a, :],
                                start=(a == 0),
                                stop=(a == 7),
                            )
                    for j in range(4):
                        nc.scalar.activation(
                            pool_sb[:, j, 0:16],
                            psp[:, j, :],
                            AF.Identity,
                            bias=bqk_sb[:, j : j + 1],
                        )

                    # chunk-sums of V_aug: cs = 128*(x_pool@wv+bv | 1) per head
                    # (66-wide psum lanes keep the fp32r dst rules satisfied)
                    psc = ps_sm.tile([NC, HPC, 66], f32, tag="cs")
                    for h in range(HPC):
                        nc.tensor.matmul(
                            psc[:, h, :],
                            ones_sb[:, 0:16],
                            bvo_sb[:, 66 * h : 66 * (h + 1)],
                            start=True,
                            stop=False,
                        )
                        for a in range(8):
                            nc.tensor.matmul(
                                psc[:, h, 0:64],
                                xtp_sb[:, a, :],
                                wv_sb[:, a, 64 * h : 64 * (h + 1)],
                                start=False,
                                stop=(a == 7),
                            )
                    for h in range(HPC):
                        nc.scalar.activation(
                            cs_sb[:, h, :], psc[:, h, 0:65], AF.Copy, scale=128.0
                        )

            # phase-B tensors (allocated after xt is freed)
            tri_sb = pers.tile([128, 128], f32, tag="tri")
            bdm_sb = pers.tile([NC, L], f32, tag="bdm")
            suf_sb = pers.tile([NC, L], bf16, tag="suf")
            t2c_sb = pers.tile([128, L], bf16, tag="t2c")
            c2t_sb = pers.tile([128, L], bf16, tag="c2t")
            oh_sb = pers.tile([128, 2, L], bf16, tag="oh")
            nc.sync.dma_start(tri_sb[:, :], d_tri[:, :])
            nc.sync.dma_start(bdm_sb[:, :], d_bdm[:, :])
            nc.sync.dma_start(suf_sb[:, :], d_suf[:, :])

            # t2c[n, l] / c2t[n, l] per head.  fp32r matmuls must write psum
            # at partition 0, so compute each head's [16, L] there, then DMA
            # SBUF->SBUF into the partition-packed layout (head h at
            # partitions 32h..32h+15) that the S_T matmuls read.
            with tc.tile_pool(
                name="ps_tc", bufs=2, space=bass.MemorySpace.PSUM
            ) as ps_tc, tc.tile_pool(name="tcs", bufs=4) as tcs:
                for h in range(HPC):
                    rb = 64 * (h % 2)
                    qblk, kblk = h // 2, 2 + h // 2
                    pst = ps_tc.tile([NC, L], f32, tag="tc")
                    psc2 = ps_tc.tile([NC, L], f32, tag="tc")
                    for p0, p1 in _split_banked(0, L):
                        nc.tensor.matmul(
                            pst[:, p0:p1],
                            pool_sb[rb : rb + 64, kblk, 0:16],
                            qk_sb[rb : rb + 64, qblk, p0:p1],
                            start=True,
                            stop=True,
                        )
                        nc.tensor.matmul(
                            psc2[:, p0:p1],
                            pool_sb[rb : rb + 64, qblk, 0:16],
                            qk_sb[rb : rb + 64, kblk, p0:p1],
                            start=True,
                            stop=True,
                        )
                    st1 = tcs.tile([NC, L], bf16, tag="tcs")
                    st2 = tcs.tile([NC, L], bf16, tag="tcs")
                    nc.scalar.activation(st1[:, :], pst[:, :], AF.Copy)
                    nc.vector.tensor_tensor(st2[:, :], psc2[:, :], bdm_sb[:, :], Alu.mult)
                    nc.sync.dma_start(t2c_sb[32 * h : 32 * h + 16, :], st1[:, :])
                    nc.sync.dma_start(c2t_sb[32 * h : 32 * h + 16, :], st2[:, :])

            # ---- phase B: per (head, l-half) score/softmax/PV ----
            with tc.tile_pool(
                name="ps_s", bufs=2, space=bass.MemorySpace.PSUM
            ) as ps_s, tc.tile_pool(
                name="ps_out", bufs=2, space=bass.MemorySpace.PSUM
            ) as ps_out, tc.tile_pool(
                name="pexp", bufs=3
            ) as pexp, tc.tile_pool(name="zp", bufs=2) as zp, tc.tile_pool(
                name="rzbp", bufs=1
            ) as rzbp:
                for h in range(HPC):
                    rb = 64 * (h % 2)
                    qblk, kblk = h // 2, 2 + h // 2
                    for half in range(2):
                        l0 = 1024 * half
                        chunks = [n for n in range(NC) if n * CH < l0 + 1024]
                        pso = ps_out.tile([65, 1024], f32, tag="out")
                        for n in chunks:
                            lstart = max(n * CH, l0)
                            span = l0 + 1024 - lstart
                            has_diag = n * CH >= l0
                            pss = ps_s.tile([128, span], f32, tag="s")
                            if has_diag:
                                nc.tensor.matmul(
                                    pss[:, 0:128],
                                    qk_sb[rb : rb + 64, kblk, n * CH : n * CH + 128],
                                    qk_sb[rb : rb + 64, qblk, n * CH : n * CH + 128],
                                    start=True,
                                    stop=True,
                                )
                                off0 = 128
                            else:
                                off0 = 0
                            for p0, p1 in _split_banked(off0, span):
                                nc.tensor.matmul(
                                    pss[:, p0:p1],
                                    c2t_sb[32 * h : 32 * h + 16, n * CH : n * CH + 128],
                                    t2c_sb[32 * h : 32 * h + 16, lstart + p0 : lstart + p1],
                                    start=True,
                                    stop=True,
                                    tile_position=(32 * h, 0),
                                )
                            if has_diag:
                                nc.vector.tensor_tensor(
                                    pss[:, 0:128], pss[:, 0:128], tri_sb[:, :], Alu.mult
                                )
                            pt = pexp.tile([128, span], bf16, tag="p")
                            nc.scalar.activation(
                                pt[:, :], pss[:, :], AF.Exp, scale=1.0 / SCALE
                            )
                            # PV accumulate
                            oofs = lstart - l0
                            first = n == chunks[0]
                            for p0, p1 in _split_banked(oofs, 1024):
                                nc.tensor.matmul(
                                    pso[:, p0:p1],
                                    v_sb[:, n, h, :],
                                    pt[:, p0 - oofs : p1 - oofs],
                                    start=first,
                                    stop=False,
                                    skip_group_check=True,
                                )
                        # masked-suffix contribution + Z count
                        for p0, p1 in _split_banked(0, 1024):
                            nc.tensor.matmul(
                                pso[:, p0:p1],
                                cs_sb[:, h, :],
                                suf_sb[:, l0 + p0 : l0 + p1],
                                start=False,
                                stop=True,
                                skip_group_check=True,
                            )
                        # 1/Z = exp(-ln Z): ln of the Z row, K=1 PE broadcast,
                        # exp(-x) evacuates the broadcast to SBUF.
                        lnz = zp.tile([1, 1024], f32r, tag="lnz")
                        nc.scalar.activation(lnz[:, :], pso[64:65, :], AF.Ln)
                        rzb = ps_s.tile([64, 1024], f32, tag="s")
                        for p0, p1 in _split_banked(0, 1024):
                            nc.tensor.matmul(
                                rzb[:, p0:p1],
                                onesr_sb[:, :],
                                lnz[:, p0:p1],
                                start=True,
                                stop=True,
                            )
                        rzb_sb = rzbp.tile([64, 1024], f32, tag="rzb")
                        nc.scalar.activation(rzb_sb[:, :], rzb[:, :], AF.Exp, scale=-1.0)
                        nc.vector.tensor_tensor(
                            oh_sb[rb : rb + 64, h // 2, l0 : l0 + 1024],
                            pso[0:64, :],
                            rzb_sb[:, :],
                            Alu.mult,
                        )

            # ---- phase C: partial o_proj ----
            out_r = d_out.rearrange("(t p) d -> p t d", p=128)
            with tc.tile_pool(
                name="ps_o", bufs=2, space=bass.MemorySpace.PSUM
            ) as ps_o, tc.tile_pool(name="outp", bufs=2) as outp:
                for lb in range(16):
                    pso2 = ps_o.tile([128, D], f32, tag="o")
                    for p0, p1 in _split_banked(0, D):
                        for j in range(2):
                            nc.tensor.matmul(
                                pso2[:, p0:p1],
                                oh_sb[:, j, 128 * lb : 128 * (lb + 1)],
                                wo_sb[:, j, p0:p1],
                                start=(j == 0),
                                stop=(j == 1),
                            )
                    ob = outp.tile([128, D], f32, tag="ob")
                    nc.scalar.activation(ob[:, 0:512], pso2[:, 0:512], AF.Copy)
                    nc.vector.tensor_copy(ob[:, 512:], pso2[:, 512:])
                    nc.sync.dma_start(out_r[:, lb, :], ob[:, :])

    _split_multiwaits(nc)
    return nc


def _host_prep(inputs):
    """Build the 8 per-core input maps."""
    x = np.asarray(inputs["x"], dtype=np.float32)
    w_qkv = np.asarray(inputs["w_qkv"], dtype=np.float32)
    b_qkv = np.asarray(inputs["b_qkv"], dtype=np.float32)
    w_o = np.asarray(inputs["w_o"], dtype=np.float32)

    tri = np.triu(np.ones((128, 128), dtype=np.float32))
    bdm = np.zeros((NC, L), dtype=np.float32)
    for n in range(NC):
        bdm[n, n * CH : (n + 1) * CH] = 1.0
    suf = np.zeros((NC, L), dtype=np.float32)
    lidx = np.arange(L) // CH
    for n in range(NC):
        suf[n, lidx < n] = 1.0

    in_maps = []
    for c in range(NCORES):
        b, g = c // HPC, c % HPC
        hs = g * HPC  # first global head
        e0 = hs * HD
        qc = slice(e0, e0 + 256)
        kc = slice(D + e0, D + e0 + 256)
        vc = slice(2 * D + e0, 2 * D + e0 + 256)
        xt = np.ascontiguousarray(x[b].T)
        xtp = np.ascontiguousarray(
            x[b].reshape(NC, CH, D).mean(axis=1, dtype=np.float32).T
        )
        wqk = np.ascontiguousarray(np.concatenate([w_qkv[:, qc], w_qkv[:, kc]], axis=1))
        bqk = np.ascontiguousarray(
            np.concatenate([b_qkv[qc], b_qkv[kc]]).reshape(4, 128).T
        )
        wv = np.ascontiguousarray(w_qkv[:, vc])
        bv = b_qkv[vc]
        bvb = np.tile(bv[None, :], (128, 1)).astype(np.float32)
        bvo = np.zeros((1, 4 * 66), dtype=np.float32)
        for h in range(HPC):
            bvo[0, 66 * h : 66 * h + 64] = bv[64 * h : 64 * (h + 1)]
            bvo[0, 66 * h + 64] = 1.0
        wo = np.ascontiguousarray(w_o[e0 : e0 + 256, :])
        in_maps.append(
            {
                "xt": xt.astype(BF16),
                "xtp": xtp.astype(BF16),
                "wqk": wqk.astype(BF16),
                "bqk": bqk,
                "wv": wv.astype(BF16),
                "bvb": bvb,
                "bvo": bvo.astype(BF16),
                "wo": wo.astype(BF16),
                "trimask": tri,
                "bdmask": bdm,
                "suf": suf.astype(BF16),
            }
        )
    return in_maps


def run_cores(inputs, trace=False, trace_kwargs=None):
    """Run the SPMD kernel; returns (per-core results, BassKernelResults)."""
    from concourse.bass_utils import run_bass_kernel_spmd

    if "nc" not in _CACHE:
        _CACHE["nc"] = _build_nc()
    nc = _CACHE["nc"]
    in_maps = _host_prep(inputs)
    res = run_bass_kernel_spmd(
        nc,
        in_maps,
        list(range(NCORES)),
        trace=trace,
        **(trace_kwargs or {}),
    )
    return res.results, res


def kernel(**inputs):
    results, _ = run_cores(inputs)
    b_o = np.asarray(inputs["b_o"], dtype=np.float32)
    out = np.zeros((B, L, D), dtype=np.float32)
    for c in range(NCORES):
        out[c // HPC] += results[c]["out"]
    out += b_o
    return out


# revision 17
# speedup vs baseline: 1.2131x; 1.0409x over previous
"""Chunked multi-head attention (CMHA) Trainium2 kernel.

Distribution: 8 NeuronCores = data-parallel over batch (2) x tensor-parallel
over heads (4 heads/core).  Each core computes, for its (batch, head-group):
qkv projection, low-rank off-diagonal scores t2c[l,n]*c2t[n,c], direct
diagonal-chunk scores, multiplicative causal mask (masked scores -> 0, so
exp(0)=1 still contributes), softmax without max-subtraction (|S|/32 <= ~2),
PV, and a partial o_proj.  Host sums the 4 partial o_proj outputs per batch
and adds b_o.

Key algebraic facts used:
  - chunk-mean pooling commutes with the qkv projection, so q_pool/k_pool/
    chunk-sums of V come from projecting a host-pooled x_pool (tiny matmuls).
  - masked (upper) region contributes exp(0)=1 weights: its PV contribution is
    128*(x_pool@Wv+bv) per masked chunk and its Z contribution is 128, both
    obtained from the pooled projection and one "suffix" matmul.
  - scores are built transposed (S_T[c, l]) so the PV matmul needs no
    transposes: lhsT = V_chunk, rhs = exp(S_T) pieces.
  - 1/Z is computed as exp(-ln Z) on the scalar engine (Ln and Exp share one
    activation table set), with the Z row broadcast across partitions by a
    K=1 PE matmul.

fp32r matmul ISA restrictions honoured: moving operand innermost count even,
dst innermost count even + 8B aligned + start_partition 0.
"""

import ml_dtypes
import numpy as np

BF16 = ml_dtypes.bfloat16

B, L, D = 2, 2048, 1024
H, HD, CH, NC = 16, 64, 128, 16  # heads, head_dim, chunk size, num chunks
HPC = 4  # heads per core
NCORES = 8
SCALE = 32.0  # sqrt(D)

_CACHE = {}


def _split_banked(lo, hi, bank=512):
    """Split [lo, hi) into pieces that do not cross `bank` boundaries."""
    out = []
    while lo < hi:
        nxt = min(hi, (lo // bank + 1) * bank)
        out.append((lo, nxt))
        lo = nxt
    return out


def _split_multiwaits(nc, limit=1):
    """Walrus in this container rejects instructions with more than one sync
    wait.  Move excess waits onto same-engine InstNoOp carriers inserted just
    before the offending instruction (engine streams execute in block order,
    so this is semantically identical)."""
    import concourse.mybir as mybir

    f = nc.m.functions[0]
    for blk in f.blocks:
        il = blk.instructions
        idx = 0
        while idx < len(il):
            inst = il[idx]
            si = getattr(inst, "sync_info", None)
            if si is None or not hasattr(si, "on_wait"):
                idx += 1
                continue
            waits = list(si.on_wait)
            if len(waits) <= limit:
                idx += 1
                continue
            eng = inst.engine
            keep, extra = waits[:limit], waits[limit:]
            nops = []
            for w in extra:
                n = nc.engines[eng].nop(hint="waitsplit").ins
                for b2 in f.blocks:
                    l2 = b2.instructions
                    for k in range(len(l2) - 1, -1, -1):
                        if l2[k] is n or l2[k].name == n.name:
                            l2.pop(k)
                            break
                n.sync_info = mybir.SyncInfo(on_wait=[w], on_update=[])
                nops.append(n)
            inst.sync_info = mybir.SyncInfo(on_wait=keep, on_update=list(si.on_update))
            for j, n in enumerate(nops):
                il.insert(idx + j, n)
            idx += len(nops) + 1
    return nc


def _build_nc():
    import concourse.bass as bass
    import concourse.mybir as mybir
    import concourse.tile as tile

    f32 = mybir.dt.float32
    f32r = mybir.dt.float32r
    bf16 = mybir.dt.bfloat16
    AF = mybir.ActivationFunctionType
    Alu = mybir.AluOpType

    nc = bass.Bass(trn_type="TRN2", target_bir_lowering=False, debug=False)

    # ---- DRAM I/O ----
    d_xt = nc.dram_tensor("xt", [D, L], bf16, kind="ExternalInput")
    d_xtp = nc.dram_tensor("xtp", [D, NC], bf16, kind="ExternalInput")
    d_wqk = nc.dram_tensor("wqk", [D, 512], bf16, kind="ExternalInput")
    d_bqk = nc.dram_tensor("bqk", [128, 4], f32, kind="ExternalInput")
    d_wv = nc.dram_tensor("wv", [D, 256], bf16, kind="ExternalInput")
    d_bvb = nc.dram_tensor("bvb", [128, 256], f32, kind="ExternalInput")
    d_bvo = nc.dram_tensor("bvo", [1, 4 * 66], bf16, kind="ExternalInput")
    d_wo = nc.dram_tensor("wo", [256, D], bf16, kind="ExternalInput")
    d_tri = nc.dram_tensor("trimask", [128, 128], f32, kind="ExternalInput")
    d_bdm = nc.dram_tensor("bdmask", [NC, L], f32, kind="ExternalInput")
    d_suf = nc.dram_tensor("suf", [NC, L], bf16, kind="ExternalInput")
    d_out = nc.dram_tensor("out", [L, D], f32, kind="ExternalOutput")

    with tile.TileContext(nc) as tc:
        with tc.tile_pool(name="pers", bufs=1) as pers:
            # ---- persistent SBUF ----
            wqk_sb = pers.tile([128, 8, 512], bf16, tag="wqk")
            wv_sb = pers.tile([128, 8, 256], bf16, tag="wv")
            wo_sb = pers.tile([128, 2, D], bf16, tag="wo")
            xtp_sb = pers.tile([128, 8, NC], bf16, tag="xtp")
            bqk_sb = pers.tile([128, 4], f32, tag="bqk")
            bvb_sb = pers.tile([128, 256], f32, tag="bvb")
            bvo_sb = pers.tile([1, 4 * 66], bf16, tag="bvo")
            qk_sb = pers.tile([128, 4, L], bf16, tag="qk")
            v_sb = pers.tile([128, NC, HPC, 65], bf16, tag="v")
            pool_sb = pers.tile([128, 4, 32], bf16, tag="pool")
            cs_sb = pers.tile([NC, HPC, 65], bf16, tag="cs")
            ones_sb = pers.tile([1, 64], bf16, tag="ones")
            onesr_sb = pers.tile([1, 64], f32r, tag="ones")

            # ---- input DMAs (xt is scoped: freed after the projections) ----
            nc.sync.dma_start(wqk_sb[:, :, :], d_wqk.rearrange("(a p) e -> p a e", p=128))
            nc.sync.dma_start(wv_sb[:, :, :], d_wv.rearrange("(a p) e -> p a e", p=128))
            nc.sync.dma_start(wo_sb[:, :, :], d_wo.rearrange("(j p) d -> p j d", p=128))
            nc.sync.dma_start(xtp_sb[:, :, :], d_xtp.rearrange("(a p) n -> p a n", p=128))
            nc.sync.dma_start(bqk_sb[:, :], d_bqk[:, :])
            nc.sync.dma_start(bvb_sb[:, :], d_bvb[:, :])
            nc.sync.dma_start(bvo_sb[:, :], d_bvo[:, :])

            nc.gpsimd.memset(ones_sb[:, :], 1.0)
            nc.gpsimd.memset(onesr_sb[:, :].bitcast(f32), 1.0)
            nc.gpsimd.memset(pool_sb[:, :, :], 0.0)
            nc.gpsimd.memset(v_sb[:, :, :, 64:65], 1.0)

            # ---- phase A: projections ----
            with tc.tile_pool(name="pxt", bufs=1) as pxt:
                xt_sb = pxt.tile([128, 8, L], bf16, tag="xt")
                xt_r = d_xt.rearrange("(a p) l -> p a l", p=128)
                for a in range(8):
                    nc.sync.dma_start(xt_sb[:, a, :], xt_r[:, a, :])

                # Q,K in [e, l] layout: out = wqk.T @ xt
                with tc.tile_pool(
                    name="ps_qk", bufs=2, space=bass.MemorySpace.PSUM
                ) as ps_qk:
                    for j in range(4):
                        ps = ps_qk.tile([128, L], f32, tag="qk")
                        for p0, p1 in _split_banked(0, L):
                            for a in range(8):
                                nc.tensor.matmul(
                                    ps[:, p0:p1],
                                    wqk_sb[:, a, 128 * j : 128 * (j + 1)],
                                    xt_sb[:, a, p0:p1],
                                    start=(a == 0),
                                    stop=(a == 7),
                                )
                        nc.vector.tensor_scalar_add(
                            qk_sb[:, j, :], ps[:, :], bqk_sb[:, j : j + 1]
                        )

                # V in [l, e] layout: out = xt.T-chunk @ wv  (+bias via bvb)
                with tc.tile_pool(
                    name="ps_v", bufs=3, space=bass.MemorySpace.PSUM
                ) as ps_v, tc.tile_pool(
                    name="ps_sm", bufs=1, space=bass.MemorySpace.PSUM
                ) as ps_sm:
                    for ci in range(NC):
                        psv = ps_v.tile([128, 256], f32, tag="v")
                        for a in range(8):
                            nc.tensor.matmul(
                                psv[:, :],
                                xt_sb[:, a, 128 * ci : 128 * (ci + 1)],
                                wv_sb[:, a, :],
                                start=(a == 0),
                                stop=(a == 7),
                            )
                        nc.vector.tensor_tensor(
                            v_sb[:, ci, :, 0:64],
                            psv.rearrange("p (h e) -> p h e", h=HPC),
                            bvb_sb.rearrange("p (h e) -> p h e", h=HPC),
                            Alu.add,
                        )

                    # pooled projections: q_pool/k_pool [e, n]
                    psp = ps_sm.tile([128, 4, NC], f32, tag="pool")
                    for j in range(4):
                        for a in range(8):
                            nc.tensor.matmul(
                                psp[:, j, :],
                                wqk_sb[:, a, 128 * j : 128 * (j + 1)],
                                xtp_sb[:, a, :],
                                start=(a == 0),
                                stop=(a == 7),
                            )
                    for j in range(4):
                        nc.scalar.activation(
                            pool_sb[:, j, 0:16],
                            psp[:, j, :],
                            AF.Identity,
                            bias=bqk_sb[:, j : j + 1],
                        )

                    # chunk-sums of V_aug: cs = 128*(x_pool@wv+bv | 1) per head
                    # (66-wide psum lanes keep the fp32r dst rules satisfied)
                    psc = ps_sm.tile([NC, HPC, 66], f32, tag="cs")
                    for h in range(HPC):
                        nc.tensor.matmul(
                            psc[:, h, :],
                            ones_sb[:, 0:16],
                            bvo_sb[:, 66 * h : 66 * (h + 1)],
                            start=True,
                            stop=False,
                        )
                        for a in range(8):
                            nc.tensor.matmul(
                                psc[:, h, 0:64],
                                xtp_sb[:, a, :],
                                wv_sb[:, a, 64 * h : 64 * (h + 1)],
                                start=False,
                                stop=(a == 7),
                            )
                    for h in range(HPC):
                        nc.scalar.activation(
                            cs_sb[:, h, :], psc[:, h, 0:65], AF.Copy, scale=128.0
                        )

            # phase-B tensors (allocated after xt is freed)
            tri_sb = pers.tile([128, 128], f32, tag="tri")
            bdm_sb = pers.tile([NC, L], f32, tag="bdm")
            suf_sb = pers.tile([NC, L], bf16, tag="suf")
            t2c_sb = pers.tile([128, L], bf16, tag="t2c")
            c2t_sb = pers.tile([128, L], bf16, tag="c2t")
            oh_sb = pers.tile([128, 2, L], bf16, tag="oh")
            nc.sync.dma_start(tri_sb[:, :], d_tri[:, :])
            nc.sync.dma_start(bdm_sb[:, :], d_bdm[:, :])
            nc.sync.dma_start(suf_sb[:, :], d_suf[:, :])

            # t2c[n, l] / c2t[n, l] per head.  fp32r matmuls must write psum
            # at partition 0, so compute each head's [16, L] there, then DMA
            # SBUF->SBUF into the partition-packed layout (head h at
            # partitions 32h..32h+15) that the S_T matmuls read.
            with tc.tile_pool(
                name="ps_tc", bufs=2, space=bass.MemorySpace.PSUM
            ) as ps_tc, tc.tile_pool(name="tcs", bufs=4) as tcs:
                for h in range(HPC):
                    rb = 64 * (h % 2)
                    qblk, kblk = h // 2, 2 + h // 2
                    pst = ps_tc.tile([NC, L], f32, tag="tc")
                    psc2 = ps_tc.tile([NC, L], f32, tag="tc")
                    for p0, p1 in _split_banked(0, L):
                        nc.tensor.matmul(
                            pst[:, p0:p1],
                            pool_sb[rb : rb + 64, kblk, 0:16],
                            qk_sb[rb : rb + 64, qblk, p0:p1],
                            start=True,
                            stop=True,
                        )
                        nc.tensor.matmul(
                            psc2[:, p0:p1],
                            pool_sb[rb : rb + 64, qblk, 0:16],
                            qk_sb[rb : rb + 64, kblk, p0:p1],
                            start=True,
                            stop=True,
                        )
                    st1 = tcs.tile([NC, L], bf16, tag="tcs")
                    st2 = tcs.tile([NC, L], bf16, tag="tcs")
                    nc.scalar.activation(st1[:, :], pst[:, :], AF.Copy)
                    nc.vector.tensor_tensor(st2[:, :], psc2[:, :], bdm_sb[:, :], Alu.mult)
                    nc.sync.dma_start(t2c_sb[32 * h : 32 * h + 16, :], st1[:, :])
                    nc.sync.dma_start(c2t_sb[32 * h : 32 * h + 16, :], st2[:, :])

            # ---- phase B: score/softmax/PV ----
            # Two heads are interleaved chunk-by-chunk so the PE never idles
            # waiting for the other engines (micro-gaps keep the HAM clock
            # gate at K=4/8 = 1.2 GHz; a dense stream runs at 2.4 GHz).
            with tc.tile_pool(
                name="ps_s", bufs=2, space=bass.MemorySpace.PSUM
            ) as ps_s, tc.tile_pool(
                name="ps_out", bufs=2, space=bass.MemorySpace.PSUM
            ) as ps_out, tc.tile_pool(
                name="pexp", bufs=4
            ) as pexp, tc.tile_pool(name="zp", bufs=2) as zp, tc.tile_pool(
                name="rzbp", bufs=2
            ) as rzbp:
                for hp in range(2):
                    for half in range(2):
                        l0 = 1024 * half
                        chunks = [n for n in range(NC) if n * CH < l0 + 1024]
                        units = [2 * hp, 2 * hp + 1]
                        psos = {}
                        for h in units:
                            pso_t = ps_out.tile([65, 1024], f32, tag="out")
                            psos[h] = pso_t
                        for n in chunks:
                            lstart = max(n * CH, l0)
                            span = l0 + 1024 - lstart
                            has_diag = n * CH >= l0
                            oofs = lstart - l0
                            first = n == chunks[0]
                            for h in units:
                                rb = 64 * (h % 2)
                                qblk, kblk = h // 2, 2 + h // 2
                                pso = psos[h]
                                pss = ps_s.tile([128, span], f32, tag="s")
                                if has_diag:
                                    nc.tensor.matmul(
                                        pss[:, 0:128],
                                        qk_sb[rb : rb + 64, kblk, n * CH : n * CH + 128],
                                        qk_sb[rb : rb + 64, qblk, n * CH : n * CH + 128],
                                        start=True,
                                        stop=True,
                                    )
                                    off0 = 128
                                else:
                                    off0 = 0
                                for p0, p1 in _split_banked(off0, span):
                                    nc.tensor.matmul(
                                        pss[:, p0:p1],
                                        c2t_sb[32 * h : 32 * h + 16, n * CH : n * CH + 128],
                                        t2c_sb[32 * h : 32 * h + 16, lstart + p0 : lstart + p1],
                                        start=True,
                                        stop=True,
                                        tile_position=(32 * h, 0),
                                    )
                                if has_diag:
                                    nc.vector.tensor_tensor(
                                        pss[:, 0:128], pss[:, 0:128], tri_sb[:, :], Alu.mult
                                    )
                                pt = pexp.tile([128, span], bf16, tag="p")
                                nc.scalar.activation(
                                    pt[:, :], pss[:, :], AF.Exp, scale=1.0 / SCALE
                                )
                                for p0, p1 in _split_banked(oofs, 1024):
                                    nc.tensor.matmul(
                                        pso[:, p0:p1],
                                        v_sb[:, n, h, :],
                                        pt[:, p0 - oofs : p1 - oofs],
                                        start=first,
                                        stop=False,
                                        skip_group_check=True,
                                    )
                        for h in units:
                            rb = 64 * (h % 2)
                            pso = psos[h]
                            # masked-suffix contribution + Z count
                            for p0, p1 in _split_banked(0, 1024):
                                nc.tensor.matmul(
                                    pso[:, p0:p1],
                                    cs_sb[:, h, :],
                                    suf_sb[:, l0 + p0 : l0 + p1],
                                    start=False,
                                    stop=True,
                                    skip_group_check=True,
                                )
                            # 1/Z = exp(-ln Z): ln of the Z row, K=1 PE
                            # broadcast, exp(-x) evacuates to SBUF.
                            lnz = zp.tile([1, 1024], f32r, tag="lnz")
                            nc.scalar.activation(lnz[:, :], pso[64:65, :], AF.Ln)
                            rzb = ps_s.tile([64, 1024], f32, tag="s")
                            for p0, p1 in _split_banked(0, 1024):
                                nc.tensor.matmul(
                                    rzb[:, p0:p1],
                                    onesr_sb[:, :],
                                    lnz[:, p0:p1],
                                    start=True,
                                    stop=True,
                                )
                            rzb_sb = rzbp.tile([64, 1024], f32, tag="rzb")
                            nc.scalar.activation(
                                rzb_sb[:, :], rzb[:, :], AF.Exp, scale=-1.0
                            )
                            nc.vector.tensor_tensor(
                                oh_sb[rb : rb + 64, h // 2, l0 : l0 + 1024],
                                pso[0:64, :],
                                rzb_sb[:, :],
                                Alu.mult,
                            )

            # ---- phase C: partial o_proj ----
            out_r = d_out.rearrange("(t p) d -> p t d", p=128)
            with tc.tile_pool(
                name="ps_o", bufs=2, space=bass.MemorySpace.PSUM
            ) as ps_o, tc.tile_pool(name="outp", bufs=2) as outp:
                for lb in range(16):
                    pso2 = ps_o.tile([128, D], f32, tag="o")
                    for p0, p1 in _split_banked(0, D):
                        for j in range(2):
                            nc.tensor.matmul(
                                pso2[:, p0:p1],
                                oh_sb[:, j, 128 * lb : 128 * (lb + 1)],
                                wo_sb[:, j, p0:p1],
                                start=(j == 0),
                                stop=(j == 1),
                            )
                    ob = outp.tile([128, D], f32, tag="ob")
                    nc.scalar.activation(ob[:, 0:512], pso2[:, 0:512], AF.Copy)
                    nc.vector.tensor_copy(ob[:, 512:], pso2[:, 512:])
                    nc.sync.dma_start(out_r[:, lb, :], ob[:, :])

    _split_multiwaits(nc)
    return nc


def _host_prep(inputs):
    """Build the 8 per-core input maps."""
    x = np.asarray(inputs["x"], dtype=np.float32)
    w_qkv = np.asarray(inputs["w_qkv"], dtype=np.float32)
    b_qkv = np.asarray(inputs["b_qkv"], dtype=np.float32)
    w_o = np.asarray(inputs["w_o"], dtype=np.float32)

    tri = np.triu(np.ones((128, 128), dtype=np.float32))
    bdm = np.zeros((NC, L), dtype=np.float32)
    for n in range(NC):
        bdm[n, n * CH : (n + 1) * CH] = 1.0
    suf = np.zeros((NC, L), dtype=np.float32)
    lidx = np.arange(L) // CH
    for n in range(NC):
        suf[n, lidx < n] = 1.0

    in_maps = []
    for c in range(NCORES):
        b, g = c // HPC, c % HPC
        hs = g * HPC  # first global head
        e0 = hs * HD
        qc = slice(e0, e0 + 256)
        kc = slice(D + e0, D + e0 + 256)
        vc = slice(2 * D + e0, 2 * D + e0 + 256)
        xt = np.ascontiguousarray(x[b].T)
        xtp = np.ascontiguousarray(
            x[b].reshape(NC, CH, D).mean(axis=1, dtype=np.float32).T
        )
        wqk = np.ascontiguousarray(np.concatenate([w_qkv[:, qc], w_qkv[:, kc]], axis=1))
        bqk = np.ascontiguousarray(
            np.concatenate([b_qkv[qc], b_qkv[kc]]).reshape(4, 128).T
        )
        wv = np.ascontiguousarray(w_qkv[:, vc])
        bv = b_qkv[vc]
        bvb = np.tile(bv[None, :], (128, 1)).astype(np.float32)
        bvo = np.zeros((1, 4 * 66), dtype=np.float32)
        for h in range(HPC):
            bvo[0, 66 * h : 66 * h + 64] = bv[64 * h : 64 * (h + 1)]
            bvo[0, 66 * h + 64] = 1.0
        wo = np.ascontiguousarray(w_o[e0 : e0 + 256, :])
        in_maps.append(
            {
                "xt": xt.astype(BF16),
                "xtp": xtp.astype(BF16),
                "wqk": wqk.astype(BF16),
                "bqk": bqk,
                "wv": wv.astype(BF16),
                "bvb": bvb,
                "bvo": bvo.astype(BF16),
                "wo": wo.astype(BF16),
                "trimask": tri,
                "bdmask": bdm,
                "suf": suf.astype(BF16),
            }
        )
    return in_maps


def run_cores(inputs, trace=False, trace_kwargs=None):
    """Run the SPMD kernel; returns (per-core results, BassKernelResults)."""
    from concourse.bass_utils import run_bass_kernel_spmd

    if "nc" not in _CACHE:
        _CACHE["nc"] = _build_nc()
    nc = _CACHE["nc"]
    in_maps = _host_prep(inputs)
    res = run_bass_kernel_spmd(
        nc,
        in_maps,
        list(range(NCORES)),
        trace=trace,
        **(trace_kwargs or {}),
    )
    return res.results, res


def kernel(**inputs):
    results, _ = run_cores(inputs)
    b_o = np.asarray(inputs["b_o"], dtype=np.float32)
    out = np.zeros((B, L, D), dtype=np.float32)
    for c in range(NCORES):
        out[c // HPC] += results[c]["out"]
    out += b_o
    return out
